# revision 11
# baseline (speedup 1.0000x reference)
"""8-core SPMD multi-head attention kernel for Trainium2 (Bass/Tile).

Problem: nn.MultiHeadAttention, B=2, S=2048, d_model=1024, 16 heads (dk=64).

Sharding: tensor-parallel over heads — 2 heads per core. Q/K/V projection
weights are column-split per core, out-projection row-split; each core
produces a partial [1024, 4096] output that the host sums (the unshard of a
row-parallel linear).

Per-core pipeline (identical SPMD program; per-core weight shards):
  1. qhT/khT/vhT projections (bf16 inputs, fp32 psum); biases are added
     per-partition during the PSUM->SBUF copy. Host pre-transposes inputs so
     the contraction dim lies on SBUF partitions.
  2. vh (token-major) from vhT via TensorE transposes; a ones column is
     appended so the attention-V matmul also yields the softmax sums.
  3. Per (batch, q-half): for each k-chunk, both heads' score matmuls are
     issued alternating tile_position row groups (K=64 pairs run
     concurrently in the PE array), exp on ACT with the 1/sqrt(dk) scale
     folded in, then V-matmuls (f32r) accumulate ctx^T plus sums.
     Normalization: reciprocal_approx_fast (DVE) + partition_broadcast
     (GpSimd) + one multiply.
  4. out_partial^T = Wo_c^T @ ctxT (f32r) + bo/8, DMA'd out.

float32r is used for on-chip matmuls: bf16-class speed, ~16x better
accuracy than bf16.
"""

import sys

sys.path.insert(0, "/opt/trn_rl_repo")

import numpy as np
import ml_dtypes

import concourse.bass as bass  # noqa: F401
import concourse.mybir as mybir
import concourse.tile as tile
from concourse import bacc
from concourse import bass_utils
from concourse.masks import make_identity

B, S, DM, H, DK = 2, 2048, 1024, 16, 64
TOK = B * S
NCORES = 8
HPC = H // NCORES    # 2 heads per core
CW = HPC * DK        # 128 = per-core qkv width
KC = DM // 128       # 8 contraction chunks
KCH = S // 128       # 16 k-token chunks per batch
F32 = mybir.dt.float32
F32R = mybir.dt.float32r
BF16 = mybir.dt.bfloat16
AFT = mybir.ActivationFunctionType

TRACE = False
LAST_EXEC_NS = None

_compiled = None
_ONES = np.ones((128, 64), np.float32)


def _build():
    nc = bacc.Bacc("TRN2", target_bir_lowering=False, debug=False,
                   num_devices=NCORES)

    qT = nc.dram_tensor("qT", [DM, TOK], BF16, kind="ExternalInput").ap()
    kT = nc.dram_tensor("kT", [DM, TOK], BF16, kind="ExternalInput").ap()
    vT = nc.dram_tensor("vT", [DM, TOK], BF16, kind="ExternalInput").ap()
    wq = nc.dram_tensor("wq", [128, KC * CW], BF16, kind="ExternalInput").ap()
    wk = nc.dram_tensor("wk", [128, KC * CW], BF16, kind="ExternalInput").ap()
    wv = nc.dram_tensor("wv", [128, KC * CW], BF16, kind="ExternalInput").ap()
    bq = nc.dram_tensor("bq", [128, 1], F32, kind="ExternalInput").ap()
    bk = nc.dram_tensor("bk", [128, 1], F32, kind="ExternalInput").ap()
    bv = nc.dram_tensor("bv", [128, 1], F32, kind="ExternalInput").ap()
    wo = nc.dram_tensor("wo", [CW, DM], F32R, kind="ExternalInput").ap()
    bo8 = nc.dram_tensor("bo8", [128, 8], F32, kind="ExternalInput").ap()
    onesd = nc.dram_tensor("onesd", [128, 64], F32R, kind="ExternalInput").ap()
    out = nc.dram_tensor("out", [DM, TOK], BF16, kind="ExternalOutput").ap()

    with tile.TileContext(nc) as tc, \
         tc.tile_pool(name="const", bufs=1) as const, \
         tc.tile_pool(name="xin", bufs=3) as xin, \
         tc.tile_pool(name="expp", bufs=2) as expp, \
         tc.tile_pool(name="stage", bufs=2) as stage, \
         tc.tile_pool(name="outst", bufs=4) as outst:

        # ---------- constants & persistent buffers ----------
        wq_sb = const.tile([128, KC, CW], BF16, tag="wq")
        wk_sb = const.tile([128, KC, CW], BF16, tag="wk")
        wv_sb = const.tile([128, KC, CW], BF16, tag="wv")
        nc.sync.dma_start(wq_sb[:], wq.rearrange("p (c m) -> p c m", c=KC))
        nc.sync.dma_start(wk_sb[:], wk.rearrange("p (c m) -> p c m", c=KC))
        nc.sync.dma_start(wv_sb[:], wv.rearrange("p (c m) -> p c m", c=KC))
        bq_sb = const.tile([128, 1], F32, tag="bq")
        bk_sb = const.tile([128, 1], F32, tag="bk")
        bv_sb = const.tile([128, 1], F32, tag="bv")
        nc.sync.dma_start(bq_sb[:], bq[:])
        nc.sync.dma_start(bk_sb[:], bk[:])
        nc.sync.dma_start(bv_sb[:], bv[:])
        wo_sb = const.tile([CW, DM], F32R, tag="wo")
        nc.sync.dma_start(wo_sb[:], wo[:])
        bo_sb = const.tile([128, 8], F32, tag="bo")
        nc.sync.dma_start(bo_sb[:], bo8[:])
        ident = const.tile([128, 128], F32, tag="ident")
        make_identity(nc, ident[:])
        ones_sb = const.tile([128, 64], F32R, tag="ones_sb")
        nc.sync.dma_start(ones_sb[:], onesd[:])

        qhT = [const.tile([128, S], F32R, tag=f"qhT{b}", name=f"qhT{b}")
               for b in range(B)]
        khT = [const.tile([128, S], F32R, tag=f"khT{b}", name=f"khT{b}")
               for b in range(B)]
        vhT = [const.tile([128, S], F32, tag=f"vhT{b}", name=f"vhT{b}")
               for b in range(B)]
        ctxT = [const.tile([128, S], F32R, tag=f"ctxT{b}", name=f"ctxT{b}")
                for b in range(B)]
        vh = const.tile([128, B, HPC, KCH, DK + 1], F32R, tag="vh")
        nc.vector.tensor_copy(
            vh[:, :, :, :, DK:DK + 1],
            ones_sb[:].rearrange("p (a b c d) -> p a b c d",
                                 a=B, b=HPC, c=KCH, d=1))

        # ---------- phase 1: projections ----------
        with tc.tile_pool(name="pproj", bufs=1, space="PSUM") as pproj:
            for name, w_sb, b_sb, xdram, dst in (
                    ("q", wq_sb, bq_sb, qT, qhT),
                    ("k", wk_sb, bk_sb, kT, khT),
                    ("v", wv_sb, bv_sb, vT, vhT)):
                ps = [pproj.tile([128, 512], F32, tag=f"pp{j}",
                                 name=f"pp_{name}{j}") for j in range(8)]
                for c in range(KC):
                    xt = xin.tile([128, TOK], BF16, tag="xt")
                    nc.sync.dma_start(xt[:], xdram[c * 128:(c + 1) * 128, :])
                    for j in range(8):
                        nc.tensor.matmul(
                            ps[j][:], w_sb[:, c, :],
                            xt[:, j * 512:(j + 1) * 512],
                            start=(c == 0), stop=(c == KC - 1))
                for j in range(8):
                    b, off = divmod(j * 512, S)
                    d = dst[b][:, off:off + 512]
                    if j % 2 == 0:
                        nc.vector.tensor_scalar_add(d, ps[j][:], b_sb[:])
                    else:
                        nc.scalar.activation(d, ps[j][:], AFT.Identity,
                                             bias=b_sb[:])

        # ---------- phase 2: vh = transpose(vhT) ----------
        with tc.tile_pool(name="ptr", bufs=2, space="PSUM") as ptr:
            for b in range(B):
                for h in range(HPC):
                    for c in range(KCH):
                        pt = ptr.tile([128, 64], F32, tag="pt")
                        hb = h * 64
                        nc.tensor.transpose(
                            pt[:],
                            vhT[b][hb:hb + 64, c * 128:(c + 1) * 128],
                            ident[hb:hb + 64, hb:hb + 64])
                        nc.vector.tensor_copy(vh[:, b, h, c, 0:DK], pt[:])

        # ---------- phase 3: attention + inline output projection ----------
        with tc.tile_pool(name="patt", bufs=1, space="PSUM") as patt:
            for b in range(B):
                for qh in range(2):          # 1024-wide q chunks
                    q0 = qh * 1024
                    pctx = [patt.tile([DK + 1, 1024], F32, tag=f"pctx{h}",
                                      name=f"pctx{b}_{qh}_{h}")
                            for h in range(HPC)]
                    for c in range(KCH):
                        pss = patt.tile([128, 2048], F32, tag="pss",
                                        name=f"pss{b}_{qh}_{c}")
                        # alternate row groups so K=64 pairs pack in the PE
                        for qn in range(2):
                            for h in range(HPC):
                                hb = h * 64
                                nc.tensor.matmul(
                                    pss[:, h * 1024 + qn * 512:
                                        h * 1024 + (qn + 1) * 512],
                                    khT[b][hb:hb + 64,
                                           c * 128:(c + 1) * 128],
                                    qhT[b][hb:hb + 64,
                                           q0 + qn * 512:q0 + (qn + 1) * 512],
                                    start=True, stop=True,
                                    tile_position=(hb, 0))
                        e = expp.tile([128, 2048], F32R, tag="exp")
                        nc.scalar.activation(e[:], pss[:], AFT.Exp,
                                             scale=0.125)
                        for h in range(HPC):
                            for qn in range(2):
                                nc.tensor.matmul(
                                    pctx[h][:, qn * 512:(qn + 1) * 512],
                                    vh[:, b, h, c, :],
                                    e[:, h * 1024 + qn * 512:
                                      h * 1024 + (qn + 1) * 512],
                                    start=(c == 0), stop=(c == KCH - 1))
                    # normalize by the softmax sums (row DK of pctx)
                    for h in range(HPC):
                        ssum = stage.tile([1, 1024], F32, tag="ssum")
                        nc.vector.tensor_copy(ssum[:], pctx[h][DK:DK + 1, :])
                        si = stage.tile([1, 1024], F32, tag="sinv")
                        nc.vector.reciprocal_approx_fast(si[:], ssum[:])
                        sbc = stage.tile([64, 1024], F32, tag="sbc")
                        nc.gpsimd.partition_broadcast(sbc[:], si[:])
                        if h == 0:
                            nc.vector.tensor_mul(
                                ctxT[b][0:64, q0:q0 + 1024],
                                pctx[h][0:64, :], sbc[:])
                        else:
                            cs = stage.tile([64, 1024], F32R, tag="cstage")
                            nc.vector.tensor_mul(
                                cs[:], pctx[h][0:64, :], sbc[:])
                            nc.sync.dma_start(
                                ctxT[b][64:128, q0:q0 + 1024], cs[:])
                    # output projection for this q-half (reuses pctx slots)
                    for od in range(8):
                        for qc in range(2):
                            po = patt.tile([128, 512], F32,
                                           tag=f"pctx{od % 2}",
                                           name=f"po{b}_{qh}_{od}_{qc}")
                            nc.tensor.matmul(
                                po[:],
                                wo_sb[:, od * 128:(od + 1) * 128],
                                ctxT[b][:, q0 + qc * 512:
                                        q0 + (qc + 1) * 512],
                                start=True, stop=True)
                            ot = outst.tile([128, 512], BF16, tag="ot")
                            if (od + qc) % 2 == 0:
                                nc.vector.tensor_scalar_add(
                                    ot[:], po[:], bo_sb[:, od:od + 1])
                            else:
                                nc.scalar.activation(
                                    ot[:], po[:], AFT.Identity,
                                    bias=bo_sb[:, od:od + 1])
                            nc.sync.dma_start(
                                out[od * 128:(od + 1) * 128,
                                    b * S + q0 + qc * 512:
                                    b * S + q0 + (qc + 1) * 512],
                                ot[:])

    nc.compile()
    return nc


def _get_compiled():
    global _compiled
    if _compiled is None:
        _compiled = _build()
    return _compiled


def _xT(x):
    xf = np.asarray(x, np.float32).reshape(TOK, DM)
    return np.ascontiguousarray(xf.T).astype(ml_dtypes.bfloat16)


def _wshuf(W, cs):
    # [1024, 128] core slice -> [p, c*128+m] so SBUF [128, KC, CW] DMAs clean
    Wc = np.asarray(W, np.float32)[:, cs]
    return np.ascontiguousarray(
        Wc.reshape(KC, 128, CW).transpose(1, 0, 2).reshape(128, KC * CW)
    ).astype(ml_dtypes.bfloat16)


def kernel(q, k, v, Wq, bq, Wk, bk, Wv, bv, Wo, bo):
    global LAST_EXEC_NS
    nc = _get_compiled()

    qTa, kTa, vTa = _xT(q), _xT(k), _xT(v)

    bq, bk, bv = (np.asarray(a, np.float32) for a in (bq, bk, bv))
    Wo = np.asarray(Wo, np.float32)
    bo = np.asarray(bo, np.float32)

    in_maps = []
    for c in range(NCORES):
        cs = slice(c * CW, (c + 1) * CW)
        in_maps.append({
            "qT": qTa, "kT": kTa, "vT": vTa,
            "wq": _wshuf(Wq, cs), "wk": _wshuf(Wk, cs), "wv": _wshuf(Wv, cs),
            "bq": np.ascontiguousarray(bq[cs].reshape(CW, 1)),
            "bk": np.ascontiguousarray(bk[cs].reshape(CW, 1)),
            "bv": np.ascontiguousarray(bv[cs].reshape(CW, 1)),
            "wo": np.ascontiguousarray(Wo[cs, :]),
            "bo8": np.ascontiguousarray((bo / NCORES).reshape(8, 128).T),
            "onesd": _ONES,
        })

    kwargs = {}
    if TRACE:
        try:
            import ntff_shim
            ntff_shim.install()
            kwargs["trace"] = True
        except Exception:
            pass

    res = bass_utils.run_bass_kernel_spmd(
        nc, in_maps, core_ids=list(range(NCORES)), **kwargs)
    LAST_EXEC_NS = res.exec_time_ns

    total = res.results[0]["out"].astype(np.float32).copy()
    for c in range(1, NCORES):
        total += res.results[c]["out"]
    return np.ascontiguousarray(total.T).reshape(B, S, DM)


# revision 12
# speedup vs baseline: 1.0506x; 1.0506x over previous
"""8-core SPMD multi-head attention kernel for Trainium2 (Bass/Tile).

Problem: nn.MultiHeadAttention, B=2, S=2048, d_model=1024, 16 heads (dk=64).

Sharding: tensor-parallel over heads — 2 heads per core. Q/K/V projection
weights are column-split per core, out-projection row-split; each core
produces a partial [1024, 4096] output that the host sums (the unshard of a
row-parallel linear).

Per-core pipeline (identical SPMD program; per-core weight shards):
  1. qhT/khT/vhT projections (bf16 inputs, fp32 psum); biases are added
     per-partition during the PSUM->SBUF copy. Host pre-transposes inputs so
     the contraction dim lies on SBUF partitions.
  2. vh (token-major) from vhT via TensorE transposes; a ones column is
     appended so the attention-V matmul also yields the softmax sums.
  3. Per (batch, q-half): for each k-chunk, both heads' score matmuls are
     issued alternating tile_position row groups (K=64 pairs run
     concurrently in the PE array), exp on ACT with the 1/sqrt(dk) scale
     folded in, then V-matmuls (f32r) accumulate ctx^T plus sums.
     Normalization: reciprocal_approx_fast (DVE) + partition_broadcast
     (GpSimd) + one multiply.
  4. out_partial^T = Wo_c^T @ ctxT (f32r) + bo/8, DMA'd out.

float32r is used for on-chip matmuls: bf16-class speed, ~16x better
accuracy than bf16.
"""

import sys

sys.path.insert(0, "/opt/trn_rl_repo")

import numpy as np
import ml_dtypes

import concourse.bass as bass  # noqa: F401
import concourse.mybir as mybir
import concourse.tile as tile
from concourse import bacc
from concourse import bass_utils
from concourse.masks import make_identity

B, S, DM, H, DK = 2, 2048, 1024, 16, 64
TOK = B * S
NCORES = 8
HPC = H // NCORES    # 2 heads per core
CW = HPC * DK        # 128 = per-core qkv width
KC = DM // 128       # 8 contraction chunks
KCH = S // 128       # 16 k-token chunks per batch
F32 = mybir.dt.float32
F32R = mybir.dt.float32r
BF16 = mybir.dt.bfloat16
AFT = mybir.ActivationFunctionType

TRACE = False
LAST_EXEC_NS = None

_compiled = None
_ONES = np.ones((128, 64), np.float32)


def _build():
    nc = bacc.Bacc("TRN2", target_bir_lowering=False, debug=False,
                   num_devices=NCORES)

    qT = nc.dram_tensor("qT", [DM, TOK], BF16, kind="ExternalInput").ap()
    kT = nc.dram_tensor("kT", [DM, TOK], BF16, kind="ExternalInput").ap()
    vT = nc.dram_tensor("vT", [DM, TOK], BF16, kind="ExternalInput").ap()
    wq = nc.dram_tensor("wq", [128, KC * CW], BF16, kind="ExternalInput").ap()
    wk = nc.dram_tensor("wk", [128, KC * CW], BF16, kind="ExternalInput").ap()
    wv = nc.dram_tensor("wv", [128, KC * CW], BF16, kind="ExternalInput").ap()
    bq = nc.dram_tensor("bq", [128, 1], F32, kind="ExternalInput").ap()
    bk = nc.dram_tensor("bk", [128, 1], F32, kind="ExternalInput").ap()
    bv = nc.dram_tensor("bv", [128, 1], F32, kind="ExternalInput").ap()
    wo = nc.dram_tensor("wo", [CW, DM], F32R, kind="ExternalInput").ap()
    bo8 = nc.dram_tensor("bo8", [128, 8], F32, kind="ExternalInput").ap()
    onesd = nc.dram_tensor("onesd", [128, 64], F32R, kind="ExternalInput").ap()
    out = nc.dram_tensor("out", [DM, TOK], BF16, kind="ExternalOutput").ap()

    with tile.TileContext(nc) as tc, \
         tc.tile_pool(name="const", bufs=1) as const, \
         tc.tile_pool(name="xin", bufs=3) as xin, \
         tc.tile_pool(name="expp", bufs=2) as expp, \
         tc.tile_pool(name="stage", bufs=2) as stage, \
         tc.tile_pool(name="outst", bufs=4) as outst:

        # ---------- constants & persistent buffers ----------
        wq_sb = const.tile([128, KC, CW], BF16, tag="wq")
        wk_sb = const.tile([128, KC, CW], BF16, tag="wk")
        wv_sb = const.tile([128, KC, CW], BF16, tag="wv")
        nc.sync.dma_start(wq_sb[:], wq.rearrange("p (c m) -> p c m", c=KC))
        nc.sync.dma_start(wk_sb[:], wk.rearrange("p (c m) -> p c m", c=KC))
        nc.sync.dma_start(wv_sb[:], wv.rearrange("p (c m) -> p c m", c=KC))
        bq_sb = const.tile([128, 1], F32, tag="bq")
        bk_sb = const.tile([128, 1], F32, tag="bk")
        bv_sb = const.tile([128, 1], F32, tag="bv")
        nc.sync.dma_start(bq_sb[:], bq[:])
        nc.sync.dma_start(bk_sb[:], bk[:])
        nc.sync.dma_start(bv_sb[:], bv[:])
        wo_sb = const.tile([CW, DM], F32R, tag="wo")
        nc.sync.dma_start(wo_sb[:], wo[:])
        bo_sb = const.tile([128, 8], F32, tag="bo")
        nc.sync.dma_start(bo_sb[:], bo8[:])
        ident = const.tile([128, 128], F32, tag="ident")
        make_identity(nc, ident[:])
        ones_sb = const.tile([128, 64], F32R, tag="ones_sb")
        nc.sync.dma_start(ones_sb[:], onesd[:])

        qhT = [const.tile([128, S], F32R, tag=f"qhT{b}", name=f"qhT{b}")
               for b in range(B)]
        khT = [const.tile([128, S], F32R, tag=f"khT{b}", name=f"khT{b}")
               for b in range(B)]
        vhT = [const.tile([128, S], F32, tag=f"vhT{b}", name=f"vhT{b}")
               for b in range(B)]
        ctxT = [const.tile([128, S], F32R, tag=f"ctxT{b}", name=f"ctxT{b}")
                for b in range(B)]
        vh = const.tile([128, B, HPC, KCH, DK + 1], F32R, tag="vh")
        nc.vector.tensor_copy(
            vh[:, :, :, :, DK:DK + 1],
            ones_sb[:].rearrange("p (a b c d) -> p a b c d",
                                 a=B, b=HPC, c=KCH, d=1))

        # ---------- phase 1: projections ----------
        with tc.tile_pool(name="pproj", bufs=1, space="PSUM") as pproj:
            for name, w_sb, b_sb, xdram, dst in (
                    ("q", wq_sb, bq_sb, qT, qhT),
                    ("k", wk_sb, bk_sb, kT, khT),
                    ("v", wv_sb, bv_sb, vT, vhT)):
                ps = [pproj.tile([128, 512], F32, tag=f"pp{j}",
                                 name=f"pp_{name}{j}") for j in range(8)]
                for c in range(KC):
                    xt = xin.tile([128, TOK], BF16, tag="xt")
                    nc.sync.dma_start(xt[:], xdram[c * 128:(c + 1) * 128, :])
                    for j in range(8):
                        nc.tensor.matmul(
                            ps[j][:], w_sb[:, c, :],
                            xt[:, j * 512:(j + 1) * 512],
                            start=(c == 0), stop=(c == KC - 1))
                for j in range(8):
                    b, off = divmod(j * 512, S)
                    d = dst[b][:, off:off + 512]
                    if j % 2 == 0:
                        nc.vector.tensor_scalar_add(d, ps[j][:], b_sb[:])
                    else:
                        nc.scalar.activation(d, ps[j][:], AFT.Identity,
                                             bias=b_sb[:])

        # ---------- phase 2: vh = transpose(vhT) ----------
        with tc.tile_pool(name="ptr", bufs=2, space="PSUM") as ptr:
            for b in range(B):
                for h in range(HPC):
                    for c in range(KCH):
                        pt = ptr.tile([128, 64], F32, tag="pt")
                        hb = h * 64
                        nc.tensor.transpose(
                            pt[:],
                            vhT[b][hb:hb + 64, c * 128:(c + 1) * 128],
                            ident[hb:hb + 64, hb:hb + 64])
                        nc.vector.tensor_copy(vh[:, b, h, c, 0:DK], pt[:])

        # ---------- phase 3: attention ----------
        with tc.tile_pool(name="patt", bufs=1, space="PSUM") as patt:
            for b in range(B):
                for qh in range(2):          # 1024-wide q chunks
                    q0 = qh * 1024
                    pctx = [patt.tile([DK + 1, 1024], F32, tag=f"pctx{h}",
                                      name=f"pctx{b}_{qh}_{h}")
                            for h in range(HPC)]
                    for c in range(KCH):
                        pss = patt.tile([128, 2048], F32, tag="pss",
                                        name=f"pss{b}_{qh}_{c}")
                        # alternate row groups so K=64 pairs pack in the PE
                        for qn in range(2):
                            for h in range(HPC):
                                hb = h * 64
                                nc.tensor.matmul(
                                    pss[:, h * 1024 + qn * 512:
                                        h * 1024 + (qn + 1) * 512],
                                    khT[b][hb:hb + 64,
                                           c * 128:(c + 1) * 128],
                                    qhT[b][hb:hb + 64,
                                           q0 + qn * 512:q0 + (qn + 1) * 512],
                                    start=True, stop=True,
                                    tile_position=(hb, 0))
                        e = expp.tile([128, 2048], F32R, tag="exp")
                        nc.scalar.activation(e[:], pss[:], AFT.Exp,
                                             scale=0.125)
                        for h in range(HPC):
                            for qn in range(2):
                                nc.tensor.matmul(
                                    pctx[h][:, qn * 512:(qn + 1) * 512],
                                    vh[:, b, h, c, :],
                                    e[:, h * 1024 + qn * 512:
                                      h * 1024 + (qn + 1) * 512],
                                    start=(c == 0), stop=(c == KCH - 1))
                    # normalize by the softmax sums (row DK of pctx)
                    for h in range(HPC):
                        ssum = stage.tile([1, 1024], F32, tag="ssum")
                        nc.vector.tensor_copy(ssum[:], pctx[h][DK:DK + 1, :])
                        si = stage.tile([1, 1024], F32, tag="sinv")
                        nc.vector.reciprocal_approx_fast(si[:], ssum[:])
                        sbc = stage.tile([64, 1024], F32, tag="sbc")
                        nc.gpsimd.partition_broadcast(sbc[:], si[:])
                        if h == 0:
                            nc.vector.tensor_mul(
                                ctxT[b][0:64, q0:q0 + 1024],
                                pctx[h][0:64, :], sbc[:])
                        else:
                            cs = stage.tile([64, 1024], F32R, tag="cstage")
                            nc.vector.tensor_mul(
                                cs[:], pctx[h][0:64, :], sbc[:])
                            nc.sync.dma_start(
                                ctxT[b][64:128, q0:q0 + 1024], cs[:])

        # ---------- phase 4: output projection (partial) ----------
        with tc.tile_pool(name="pout", bufs=4, space="PSUM") as pout:
            for b in range(B):
                for od in range(8):
                    for qc in range(4):
                        po = pout.tile([128, 512], F32, tag="po")
                        nc.tensor.matmul(
                            po[:],
                            wo_sb[:, od * 128:(od + 1) * 128],
                            ctxT[b][:, qc * 512:(qc + 1) * 512],
                            start=True, stop=True)
                        ot = outst.tile([128, 512], BF16, tag="ot")
                        if (od + qc) % 2 == 0:
                            nc.vector.tensor_scalar_add(
                                ot[:], po[:], bo_sb[:, od:od + 1])
                        else:
                            nc.scalar.activation(ot[:], po[:], AFT.Identity,
                                                 bias=bo_sb[:, od:od + 1])
                        nc.sync.dma_start(
                            out[od * 128:(od + 1) * 128,
                                b * S + qc * 512:b * S + (qc + 1) * 512],
                            ot[:])

    nc.compile()
    return nc


def _get_compiled():
    global _compiled
    if _compiled is None:
        _compiled = _build()
    return _compiled


def _xT(x):
    xf = np.asarray(x, np.float32).reshape(TOK, DM)
    return np.ascontiguousarray(xf.T).astype(ml_dtypes.bfloat16)


def _wshuf(W, cs):
    # [1024, 128] core slice -> [p, c*128+m] so SBUF [128, KC, CW] DMAs clean
    Wc = np.asarray(W, np.float32)[:, cs]
    return np.ascontiguousarray(
        Wc.reshape(KC, 128, CW).transpose(1, 0, 2).reshape(128, KC * CW)
    ).astype(ml_dtypes.bfloat16)


def kernel(q, k, v, Wq, bq, Wk, bk, Wv, bv, Wo, bo):
    global LAST_EXEC_NS
    nc = _get_compiled()

    qTa, kTa, vTa = _xT(q), _xT(k), _xT(v)

    bq, bk, bv = (np.asarray(a, np.float32) for a in (bq, bk, bv))
    Wo = np.asarray(Wo, np.float32)
    bo = np.asarray(bo, np.float32)

    in_maps = []
    for c in range(NCORES):
        cs = slice(c * CW, (c + 1) * CW)
        in_maps.append({
            "qT": qTa, "kT": kTa, "vT": vTa,
            "wq": _wshuf(Wq, cs), "wk": _wshuf(Wk, cs), "wv": _wshuf(Wv, cs),
            "bq": np.ascontiguousarray(bq[cs].reshape(CW, 1)),
            "bk": np.ascontiguousarray(bk[cs].reshape(CW, 1)),
            "bv": np.ascontiguousarray(bv[cs].reshape(CW, 1)),
            "wo": np.ascontiguousarray(Wo[cs, :]),
            "bo8": np.ascontiguousarray((bo / NCORES).reshape(8, 128).T),
            "onesd": _ONES,
        })

    kwargs = {}
    if TRACE:
        try:
            import ntff_shim
            ntff_shim.install()
            kwargs["trace"] = True
        except Exception:
            pass

    res = bass_utils.run_bass_kernel_spmd(
        nc, in_maps, core_ids=list(range(NCORES)), **kwargs)
    LAST_EXEC_NS = res.exec_time_ns

    total = res.results[0]["out"].astype(np.float32).copy()
    for c in range(1, NCORES):
        total += res.results[c]["out"]
    return np.ascontiguousarray(total.T).reshape(B, S, DM)


# revision 13
# speedup vs baseline: 1.3907x; 1.3237x over previous
"""8-core SPMD multi-head attention kernel for Trainium2 (Bass/Tile).

Problem: nn.MultiHeadAttention, B=2, S=2048, d_model=1024, 16 heads (dk=64).

Sharding: tensor-parallel over heads — 2 heads per core. Q/K/V projection
weights are column-split per core, out-projection row-split; each core
produces a partial [1024, 4096] output that the host sums (the unshard of a
row-parallel linear).

Per-core pipeline (identical SPMD program; per-core weight shards):
  1. qhT/khT/vhT projections (bf16 inputs, fp32 psum); biases are added
     per-partition during the PSUM->SBUF copy. Host pre-transposes inputs so
     the contraction dim lies on SBUF partitions.
  2. vh (token-major) from vhT via TensorE transposes; a ones column is
     appended so the attention-V matmul also yields the softmax sums.
  3. Per (batch, q-half): for each k-chunk, both heads' score matmuls are
     issued alternating tile_position row groups (K=64 pairs run
     concurrently in the PE array), exp on ACT with the 1/sqrt(dk) scale
     folded in, then V-matmuls (f32r) accumulate ctx^T plus sums.
     Normalization: reciprocal_approx_fast (DVE) + partition_broadcast
     (GpSimd) + one multiply.
  4. out_partial^T = Wo_c^T @ ctxT (f32r) + bo/8, DMA'd out.

float32r is used for on-chip matmuls: bf16-class speed, ~16x better
accuracy than bf16.
"""

import sys

sys.path.insert(0, "/opt/trn_rl_repo")

import numpy as np
import ml_dtypes

import concourse.bass as bass  # noqa: F401
import concourse.mybir as mybir
import concourse.tile as tile
from concourse import bacc
from concourse import bass_utils
from concourse.masks import make_identity

B, S, DM, H, DK = 2, 2048, 1024, 16, 64
TOK = B * S
NCORES = 8
HPC = H // NCORES    # 2 heads per core
CW = HPC * DK        # 128 = per-core qkv width
KC = DM // 128       # 8 contraction chunks
KCH = S // 128       # 16 k-token chunks per batch
F32 = mybir.dt.float32
F32R = mybir.dt.float32r
BF16 = mybir.dt.bfloat16
AFT = mybir.ActivationFunctionType

TRACE = False
LAST_EXEC_NS = None

_compiled = None
_ONES = np.ones((128, 64), np.float32)


def _build():
    nc = bacc.Bacc("TRN2", target_bir_lowering=False, debug=False,
                   num_devices=NCORES)

    qT = nc.dram_tensor("qT", [DM, TOK], BF16, kind="ExternalInput").ap()
    kT = nc.dram_tensor("kT", [DM, TOK], BF16, kind="ExternalInput").ap()
    vT = nc.dram_tensor("vT", [DM, TOK], BF16, kind="ExternalInput").ap()
    wq = nc.dram_tensor("wq", [128, KC * CW], BF16, kind="ExternalInput").ap()
    wk = nc.dram_tensor("wk", [128, KC * CW], BF16, kind="ExternalInput").ap()
    wv = nc.dram_tensor("wv", [128, KC * CW], BF16, kind="ExternalInput").ap()
    bq = nc.dram_tensor("bq", [128, 1], F32, kind="ExternalInput").ap()
    bk = nc.dram_tensor("bk", [128, 1], F32, kind="ExternalInput").ap()
    bv = nc.dram_tensor("bv", [128, 1], F32, kind="ExternalInput").ap()
    wo = nc.dram_tensor("wo", [CW, DM], F32R, kind="ExternalInput").ap()
    bo8 = nc.dram_tensor("bo8", [128, 8], F32, kind="ExternalInput").ap()
    onesd = nc.dram_tensor("onesd", [128, 64], F32R, kind="ExternalInput").ap()
    out = nc.dram_tensor("out", [DM, TOK], BF16, kind="ExternalOutput").ap()

    with tile.TileContext(nc) as tc, \
         tc.tile_pool(name="const", bufs=1) as const, \
         tc.tile_pool(name="xin", bufs=3) as xin, \
         tc.tile_pool(name="expp", bufs=2) as expp, \
         tc.tile_pool(name="stage", bufs=2) as stage, \
         tc.tile_pool(name="outst", bufs=4) as outst:

        # ---------- constants & persistent buffers ----------
        wq_sb = const.tile([128, KC, CW], BF16, tag="wq")
        wk_sb = const.tile([128, KC, CW], BF16, tag="wk")
        wv_sb = const.tile([128, KC, CW], BF16, tag="wv")
        nc.sync.dma_start(wq_sb[:], wq.rearrange("p (c m) -> p c m", c=KC))
        nc.sync.dma_start(wk_sb[:], wk.rearrange("p (c m) -> p c m", c=KC))
        nc.sync.dma_start(wv_sb[:], wv.rearrange("p (c m) -> p c m", c=KC))
        bq_sb = const.tile([128, 1], F32, tag="bq")
        bk_sb = const.tile([128, 1], F32, tag="bk")
        bv_sb = const.tile([128, 1], F32, tag="bv")
        nc.sync.dma_start(bq_sb[:], bq[:])
        nc.sync.dma_start(bk_sb[:], bk[:])
        nc.sync.dma_start(bv_sb[:], bv[:])
        wo_sb = const.tile([CW, DM], F32R, tag="wo")
        nc.sync.dma_start(wo_sb[:], wo[:])
        bo_sb = const.tile([128, 8], F32, tag="bo")
        nc.sync.dma_start(bo_sb[:], bo8[:])
        ident = const.tile([128, 128], F32, tag="ident")
        make_identity(nc, ident[:])
        ones_sb = const.tile([128, 64], F32R, tag="ones_sb")
        nc.sync.dma_start(ones_sb[:], onesd[:])

        qhT = [const.tile([128, S], F32R, tag=f"qhT{b}", name=f"qhT{b}")
               for b in range(B)]
        khT = [const.tile([128, S], F32R, tag=f"khT{b}", name=f"khT{b}")
               for b in range(B)]
        vhT = [const.tile([128, S], F32, tag=f"vhT{b}", name=f"vhT{b}")
               for b in range(B)]
        ctxT = [const.tile([128, S], F32R, tag=f"ctxT{b}", name=f"ctxT{b}")
                for b in range(B)]
        vh = const.tile([128, B, HPC, KCH, DK + 1], F32R, tag="vh")
        nc.vector.tensor_copy(
            vh[:, :, :, :, DK:DK + 1],
            ones_sb[:].rearrange("p (a b c d) -> p a b c d",
                                 a=B, b=HPC, c=KCH, d=1))

        # ---------- phase 1: projections ----------
        with tc.tile_pool(name="pproj", bufs=1, space="PSUM") as pproj:
            for name, w_sb, b_sb, xdram, dst in (
                    ("q", wq_sb, bq_sb, qT, qhT),
                    ("k", wk_sb, bk_sb, kT, khT),
                    ("v", wv_sb, bv_sb, vT, vhT)):
                ps = [pproj.tile([128, 512], F32, tag=f"pp{j}",
                                 name=f"pp_{name}{j}") for j in range(8)]
                for c in range(KC):
                    xt = xin.tile([128, TOK], BF16, tag="xt")
                    nc.sync.dma_start(xt[:], xdram[c * 128:(c + 1) * 128, :])
                    for j in range(8):
                        nc.tensor.matmul(
                            ps[j][:], w_sb[:, c, :],
                            xt[:, j * 512:(j + 1) * 512],
                            start=(c == 0), stop=(c == KC - 1))
                for j in range(8):
                    b, off = divmod(j * 512, S)
                    d = dst[b][:, off:off + 512]
                    if j % 2 == 0:
                        nc.vector.tensor_scalar_add(d, ps[j][:], b_sb[:])
                    else:
                        nc.scalar.activation(d, ps[j][:], AFT.Identity,
                                             bias=b_sb[:])

        # ---------- phase 2: vh = transpose(vhT) ----------
        with tc.tile_pool(name="ptr", bufs=2, space="PSUM") as ptr:
            for b in range(B):
                for h in range(HPC):
                    for c in range(KCH):
                        pt = ptr.tile([128, 64], F32, tag="pt")
                        hb = h * 64
                        nc.tensor.transpose(
                            pt[:],
                            vhT[b][hb:hb + 64, c * 128:(c + 1) * 128],
                            ident[hb:hb + 64, hb:hb + 64])
                        nc.vector.tensor_copy(vh[:, b, h, c, 0:DK], pt[:])

        # ---------- phase 3: attention ----------
        with tc.tile_pool(name="patt", bufs=1, space="PSUM") as patt:
            for b in range(B):
                for qh in range(2):          # 1024-wide q chunks
                    q0 = qh * 1024
                    pctx = [patt.tile([DK + 1, 1024], F32, tag=f"pctx{h}",
                                      name=f"pctx{b}_{qh}_{h}")
                            for h in range(HPC)]
                    for c in range(KCH):
                        pss = [patt.tile([128, 1024], F32, tag=f"pss{h}",
                                         name=f"pss{b}_{qh}_{c}_{h}")
                               for h in range(HPC)]
                        # alternate row groups so K=64 pairs pack in the PE
                        for qn in range(2):
                            for h in range(HPC):
                                hb = h * 64
                                nc.tensor.matmul(
                                    pss[h][:, qn * 512:(qn + 1) * 512],
                                    khT[b][hb:hb + 64,
                                           c * 128:(c + 1) * 128],
                                    qhT[b][hb:hb + 64,
                                           q0 + qn * 512:q0 + (qn + 1) * 512],
                                    start=True, stop=True,
                                    tile_position=(hb, 0))
                        ets = []
                        for h in range(HPC):
                            e = expp.tile([128, 1024], F32R, tag=f"exp{h}")
                            nc.scalar.activation(e[:], pss[h][:], AFT.Exp,
                                                 scale=0.125)
                            ets.append(e)
                        for h in range(HPC):
                            for qn in range(2):
                                nc.tensor.matmul(
                                    pctx[h][:, qn * 512:(qn + 1) * 512],
                                    vh[:, b, h, c, :],
                                    ets[h][:, qn * 512:(qn + 1) * 512],
                                    start=(c == 0), stop=(c == KCH - 1))
                    # normalize by the softmax sums (row DK of pctx)
                    for h in range(HPC):
                        ssum = stage.tile([1, 1024], F32, tag="ssum")
                        nc.vector.tensor_copy(ssum[:], pctx[h][DK:DK + 1, :])
                        si = stage.tile([1, 1024], F32, tag="sinv")
                        nc.vector.reciprocal_approx_fast(si[:], ssum[:])
                        sbc = stage.tile([64, 1024], F32, tag="sbc")
                        nc.gpsimd.partition_broadcast(sbc[:], si[:])
                        if h == 0:
                            nc.vector.tensor_mul(
                                ctxT[b][0:64, q0:q0 + 1024],
                                pctx[h][0:64, :], sbc[:])
                        else:
                            cs = stage.tile([64, 1024], F32R, tag="cstage")
                            nc.vector.tensor_mul(
                                cs[:], pctx[h][0:64, :], sbc[:])
                            nc.sync.dma_start(
                                ctxT[b][64:128, q0:q0 + 1024], cs[:])

        # ---------- phase 4: output projection (partial) ----------
        with tc.tile_pool(name="pout", bufs=4, space="PSUM") as pout:
            for b in range(B):
                for od in range(8):
                    for qc in range(4):
                        po = pout.tile([128, 512], F32, tag="po")
                        nc.tensor.matmul(
                            po[:],
                            wo_sb[:, od * 128:(od + 1) * 128],
                            ctxT[b][:, qc * 512:(qc + 1) * 512],
                            start=True, stop=True)
                        ot = outst.tile([128, 512], BF16, tag="ot")
                        if (od + qc) % 2 == 0:
                            nc.vector.tensor_scalar_add(
                                ot[:], po[:], bo_sb[:, od:od + 1])
                        else:
                            nc.scalar.activation(ot[:], po[:], AFT.Identity,
                                                 bias=bo_sb[:, od:od + 1])
                        nc.sync.dma_start(
                            out[od * 128:(od + 1) * 128,
                                b * S + qc * 512:b * S + (qc + 1) * 512],
                            ot[:])

    nc.compile()
    return nc


def _get_compiled():
    global _compiled
    if _compiled is None:
        _compiled = _build()
    return _compiled


def _xT(x):
    xf = np.asarray(x, np.float32).reshape(TOK, DM)
    return np.ascontiguousarray(xf.T).astype(ml_dtypes.bfloat16)


def _wshuf(W, cs):
    # [1024, 128] core slice -> [p, c*128+m] so SBUF [128, KC, CW] DMAs clean
    Wc = np.asarray(W, np.float32)[:, cs]
    return np.ascontiguousarray(
        Wc.reshape(KC, 128, CW).transpose(1, 0, 2).reshape(128, KC * CW)
    ).astype(ml_dtypes.bfloat16)


def kernel(q, k, v, Wq, bq, Wk, bk, Wv, bv, Wo, bo):
    global LAST_EXEC_NS
    nc = _get_compiled()

    qTa, kTa, vTa = _xT(q), _xT(k), _xT(v)

    bq, bk, bv = (np.asarray(a, np.float32) for a in (bq, bk, bv))
    Wo = np.asarray(Wo, np.float32)
    bo = np.asarray(bo, np.float32)

    in_maps = []
    for c in range(NCORES):
        cs = slice(c * CW, (c + 1) * CW)
        in_maps.append({
            "qT": qTa, "kT": kTa, "vT": vTa,
            "wq": _wshuf(Wq, cs), "wk": _wshuf(Wk, cs), "wv": _wshuf(Wv, cs),
            "bq": np.ascontiguousarray(bq[cs].reshape(CW, 1)),
            "bk": np.ascontiguousarray(bk[cs].reshape(CW, 1)),
            "bv": np.ascontiguousarray(bv[cs].reshape(CW, 1)),
            "wo": np.ascontiguousarray(Wo[cs, :]),
            "bo8": np.ascontiguousarray((bo / NCORES).reshape(8, 128).T),
            "onesd": _ONES,
        })

    kwargs = {}
    if TRACE:
        try:
            import ntff_shim
            ntff_shim.install()
            kwargs["trace"] = True
        except Exception:
            pass

    res = bass_utils.run_bass_kernel_spmd(
        nc, in_maps, core_ids=list(range(NCORES)), **kwargs)
    LAST_EXEC_NS = res.exec_time_ns

    total = res.results[0]["out"].astype(np.float32).copy()
    for c in range(1, NCORES):
        total += res.results[c]["out"]
    return np.ascontiguousarray(total.T).reshape(B, S, DM)


# revision 14
# speedup vs baseline: 1.4016x; 1.0078x over previous
"""8-core SPMD multi-head attention kernel for Trainium2 (Bass/Tile).

Problem: nn.MultiHeadAttention, B=2, S=2048, d_model=1024, 16 heads (dk=64).

Sharding: tensor-parallel over heads — 2 heads per core. Q/K/V projection
weights are column-split per core, out-projection row-split; each core
produces a partial [1024, 4096] output that the host sums (the unshard of a
row-parallel linear).

Per-core pipeline (identical SPMD program; per-core weight shards):
  1. qhT/khT/vhT projections (bf16 inputs, fp32 psum); biases are added
     per-partition during the PSUM->SBUF copy. Host pre-transposes inputs so
     the contraction dim lies on SBUF partitions.
  2. vh (token-major) from vhT via TensorE transposes; a ones column is
     appended so the attention-V matmul also yields the softmax sums.
  3. Per (batch, q-half): for each k-chunk, both heads' score matmuls are
     issued alternating tile_position row groups (K=64 pairs run
     concurrently in the PE array), exp on ACT with the 1/sqrt(dk) scale
     folded in, then V-matmuls (f32r) accumulate ctx^T plus sums.
     Normalization: reciprocal_approx_fast (DVE) + partition_broadcast
     (GpSimd) + one multiply.
  4. out_partial^T = Wo_c^T @ ctxT (f32r) + bo/8, DMA'd out.

float32r is used for on-chip matmuls: bf16-class speed, ~16x better
accuracy than bf16.
"""

import sys

sys.path.insert(0, "/opt/trn_rl_repo")

import numpy as np
import ml_dtypes

import concourse.bass as bass  # noqa: F401
import concourse.mybir as mybir
import concourse.tile as tile
from concourse import bacc
from concourse import bass_utils
from concourse.masks import make_identity

B, S, DM, H, DK = 2, 2048, 1024, 16, 64
TOK = B * S
NCORES = 8
HPC = H // NCORES    # 2 heads per core
CW = HPC * DK        # 128 = per-core qkv width
KC = DM // 128       # 8 contraction chunks
KCH = S // 128       # 16 k-token chunks per batch
F32 = mybir.dt.float32
F32R = mybir.dt.float32r
BF16 = mybir.dt.bfloat16
AFT = mybir.ActivationFunctionType

TRACE = False
LAST_EXEC_NS = None

_compiled = None
_ONES = np.ones((128, 64), np.float32)


def _build():
    nc = bacc.Bacc("TRN2", target_bir_lowering=False, debug=False,
                   num_devices=NCORES)

    qT = nc.dram_tensor("qT", [DM, TOK], BF16, kind="ExternalInput").ap()
    kT = nc.dram_tensor("kT", [DM, TOK], BF16, kind="ExternalInput").ap()
    vT = nc.dram_tensor("vT", [DM, TOK], BF16, kind="ExternalInput").ap()
    wq = nc.dram_tensor("wq", [128, KC * CW], BF16, kind="ExternalInput").ap()
    wk = nc.dram_tensor("wk", [128, KC * CW], BF16, kind="ExternalInput").ap()
    wv = nc.dram_tensor("wv", [128, KC * CW], BF16, kind="ExternalInput").ap()
    bq = nc.dram_tensor("bq", [128, 1], F32, kind="ExternalInput").ap()
    bk = nc.dram_tensor("bk", [128, 1], F32, kind="ExternalInput").ap()
    bv = nc.dram_tensor("bv", [128, 1], F32, kind="ExternalInput").ap()
    wo = nc.dram_tensor("wo", [CW, DM], F32R, kind="ExternalInput").ap()
    bo8 = nc.dram_tensor("bo8", [128, 8], F32, kind="ExternalInput").ap()
    onesd = nc.dram_tensor("onesd", [128, 64], F32R, kind="ExternalInput").ap()
    out = nc.dram_tensor("out", [DM, TOK], BF16, kind="ExternalOutput").ap()

    with tile.TileContext(nc) as tc, \
         tc.tile_pool(name="const", bufs=1) as const, \
         tc.tile_pool(name="xin", bufs=3) as xin, \
         tc.tile_pool(name="expp", bufs=2) as expp, \
         tc.tile_pool(name="stage", bufs=2) as stage, \
         tc.tile_pool(name="outst", bufs=4) as outst:

        # ---------- constants & persistent buffers ----------
        wq_sb = const.tile([128, KC, CW], BF16, tag="wq")
        wk_sb = const.tile([128, KC, CW], BF16, tag="wk")
        wv_sb = const.tile([128, KC, CW], BF16, tag="wv")
        nc.sync.dma_start(wq_sb[:], wq.rearrange("p (c m) -> p c m", c=KC))
        nc.sync.dma_start(wk_sb[:], wk.rearrange("p (c m) -> p c m", c=KC))
        nc.sync.dma_start(wv_sb[:], wv.rearrange("p (c m) -> p c m", c=KC))
        bq_sb = const.tile([128, 1], F32, tag="bq")
        bk_sb = const.tile([128, 1], F32, tag="bk")
        bv_sb = const.tile([128, 1], F32, tag="bv")
        nc.sync.dma_start(bq_sb[:], bq[:])
        nc.sync.dma_start(bk_sb[:], bk[:])
        nc.sync.dma_start(bv_sb[:], bv[:])
        wo_sb = const.tile([CW, DM], F32R, tag="wo")
        nc.sync.dma_start(wo_sb[:], wo[:])
        bo_sb = const.tile([128, 8], F32, tag="bo")
        nc.sync.dma_start(bo_sb[:], bo8[:])
        ident = const.tile([128, 128], F32, tag="ident")
        make_identity(nc, ident[:])
        ones_sb = const.tile([128, 64], F32R, tag="ones_sb")
        nc.sync.dma_start(ones_sb[:], onesd[:])

        qhT = [const.tile([128, S], F32R, tag=f"qhT{b}", name=f"qhT{b}")
               for b in range(B)]
        khT = [const.tile([128, S], F32R, tag=f"khT{b}", name=f"khT{b}")
               for b in range(B)]
        vhT = [const.tile([128, S], F32, tag=f"vhT{b}", name=f"vhT{b}")
               for b in range(B)]
        ctxT = [const.tile([128, S], F32R, tag=f"ctxT{b}", name=f"ctxT{b}")
                for b in range(B)]
        vh = const.tile([128, B, HPC, KCH, DK + 1], F32R, tag="vh")
        nc.vector.tensor_copy(
            vh[:, :, :, :, DK:DK + 1],
            ones_sb[:].rearrange("p (a b c d) -> p a b c d",
                                 a=B, b=HPC, c=KCH, d=1))

        # ---------- phase 1: projections ----------
        with tc.tile_pool(name="pproj", bufs=1, space="PSUM") as pproj:
            for name, w_sb, b_sb, xdram, dst in (
                    ("q", wq_sb, bq_sb, qT, qhT),
                    ("k", wk_sb, bk_sb, kT, khT),
                    ("v", wv_sb, bv_sb, vT, vhT)):
                ps = [pproj.tile([128, 512], F32, tag=f"pp{j}",
                                 name=f"pp_{name}{j}") for j in range(8)]
                for c in range(KC):
                    xt = xin.tile([128, TOK], BF16, tag="xt")
                    nc.sync.dma_start(xt[:, 0:TOK // 2],
                                      xdram[c * 128:(c + 1) * 128, 0:TOK // 2])
                    nc.sync.dma_start(xt[:, TOK // 2:TOK],
                                      xdram[c * 128:(c + 1) * 128, TOK // 2:TOK])
                    for j in range(8):
                        nc.tensor.matmul(
                            ps[j][:], w_sb[:, c, :],
                            xt[:, j * 512:(j + 1) * 512],
                            start=(c == 0), stop=(c == KC - 1))
                for j in range(8):
                    b, off = divmod(j * 512, S)
                    d = dst[b][:, off:off + 512]
                    if j % 2 == 0:
                        nc.vector.tensor_scalar_add(d, ps[j][:], b_sb[:])
                    else:
                        nc.scalar.activation(d, ps[j][:], AFT.Identity,
                                             bias=b_sb[:])

        # ---------- phase 2: vh = transpose(vhT) ----------
        with tc.tile_pool(name="ptr", bufs=2, space="PSUM") as ptr:
            for b in range(B):
                for h in range(HPC):
                    for c in range(KCH):
                        pt = ptr.tile([128, 64], F32, tag="pt")
                        hb = h * 64
                        nc.tensor.transpose(
                            pt[:],
                            vhT[b][hb:hb + 64, c * 128:(c + 1) * 128],
                            ident[hb:hb + 64, hb:hb + 64])
                        nc.vector.tensor_copy(vh[:, b, h, c, 0:DK], pt[:])

        # ---------- phase 3: attention ----------
        with tc.tile_pool(name="patt", bufs=1, space="PSUM") as patt:
            for b in range(B):
                for qh in range(2):          # 1024-wide q chunks
                    q0 = qh * 1024
                    pctx = [patt.tile([DK + 1, 1024], F32, tag=f"pctx{h}",
                                      name=f"pctx{b}_{qh}_{h}")
                            for h in range(HPC)]
                    def do_av(c, ets):
                        for h in range(HPC):
                            for qn in range(2):
                                nc.tensor.matmul(
                                    pctx[h][:, qn * 512:(qn + 1) * 512],
                                    vh[:, b, h, c, :],
                                    ets[h][:, qn * 512:(qn + 1) * 512],
                                    start=(c == 0), stop=(c == KCH - 1))

                    pend = None
                    for c in range(KCH):
                        pss = [patt.tile([128, 1024], F32, tag=f"pss{h}",
                                         name=f"pss{b}_{qh}_{c}_{h}")
                               for h in range(HPC)]
                        # alternate row groups so K=64 pairs pack in the PE
                        for qn in range(2):
                            for h in range(HPC):
                                hb = h * 64
                                nc.tensor.matmul(
                                    pss[h][:, qn * 512:(qn + 1) * 512],
                                    khT[b][hb:hb + 64,
                                           c * 128:(c + 1) * 128],
                                    qhT[b][hb:hb + 64,
                                           q0 + qn * 512:q0 + (qn + 1) * 512],
                                    start=True, stop=True,
                                    tile_position=(hb, 0))
                        if pend is not None:
                            do_av(*pend)
                        ets = []
                        for h in range(HPC):
                            e = expp.tile([128, 1024], F32R, tag=f"exp{h}")
                            nc.scalar.activation(e[:], pss[h][:], AFT.Exp,
                                                 scale=0.125)
                            ets.append(e)
                        pend = (c, ets)
                    do_av(*pend)
                    # normalize by the softmax sums (row DK of pctx)
                    for h in range(HPC):
                        ssum = stage.tile([1, 1024], F32, tag="ssum")
                        nc.vector.tensor_copy(ssum[:], pctx[h][DK:DK + 1, :])
                        si = stage.tile([1, 1024], F32, tag="sinv")
                        nc.vector.reciprocal_approx_fast(si[:], ssum[:])
                        sbc = stage.tile([64, 1024], F32, tag="sbc")
                        nc.gpsimd.partition_broadcast(sbc[:], si[:])
                        if h == 0:
                            nc.vector.tensor_mul(
                                ctxT[b][0:64, q0:q0 + 1024],
                                pctx[h][0:64, :], sbc[:])
                        else:
                            cs = stage.tile([64, 1024], F32R, tag="cstage")
                            nc.vector.tensor_mul(
                                cs[:], pctx[h][0:64, :], sbc[:])
                            nc.sync.dma_start(
                                ctxT[b][64:128, q0:q0 + 1024], cs[:])

        # ---------- phase 4: output projection (partial) ----------
        with tc.tile_pool(name="pout", bufs=4, space="PSUM") as pout:
            for b in range(B):
                for od in range(8):
                    for qc in range(4):
                        po = pout.tile([128, 512], F32, tag="po")
                        nc.tensor.matmul(
                            po[:],
                            wo_sb[:, od * 128:(od + 1) * 128],
                            ctxT[b][:, qc * 512:(qc + 1) * 512],
                            start=True, stop=True)
                        ot = outst.tile([128, 512], BF16, tag="ot")
                        if (od + qc) % 2 == 0:
                            nc.vector.tensor_scalar_add(
                                ot[:], po[:], bo_sb[:, od:od + 1])
                        else:
                            nc.scalar.activation(ot[:], po[:], AFT.Identity,
                                                 bias=bo_sb[:, od:od + 1])
                        nc.sync.dma_start(
                            out[od * 128:(od + 1) * 128,
                                b * S + qc * 512:b * S + (qc + 1) * 512],
                            ot[:])

    nc.compile()
    return nc


def _get_compiled():
    global _compiled
    if _compiled is None:
        _compiled = _build()
    return _compiled


def _xT(x):
    xf = np.asarray(x, np.float32).reshape(TOK, DM)
    return np.ascontiguousarray(xf.T).astype(ml_dtypes.bfloat16)


def _wshuf(W, cs):
    # [1024, 128] core slice -> [p, c*128+m] so SBUF [128, KC, CW] DMAs clean
    Wc = np.asarray(W, np.float32)[:, cs]
    return np.ascontiguousarray(
        Wc.reshape(KC, 128, CW).transpose(1, 0, 2).reshape(128, KC * CW)
    ).astype(ml_dtypes.bfloat16)


def kernel(q, k, v, Wq, bq, Wk, bk, Wv, bv, Wo, bo):
    global LAST_EXEC_NS
    nc = _get_compiled()

    qTa, kTa, vTa = _xT(q), _xT(k), _xT(v)

    bq, bk, bv = (np.asarray(a, np.float32) for a in (bq, bk, bv))
    Wo = np.asarray(Wo, np.float32)
    bo = np.asarray(bo, np.float32)

    in_maps = []
    for c in range(NCORES):
        cs = slice(c * CW, (c + 1) * CW)
        in_maps.append({
            "qT": qTa, "kT": kTa, "vT": vTa,
            "wq": _wshuf(Wq, cs), "wk": _wshuf(Wk, cs), "wv": _wshuf(Wv, cs),
            "bq": np.ascontiguousarray(bq[cs].reshape(CW, 1)),
            "bk": np.ascontiguousarray(bk[cs].reshape(CW, 1)),
            "bv": np.ascontiguousarray(bv[cs].reshape(CW, 1)),
            "wo": np.ascontiguousarray(Wo[cs, :]),
            "bo8": np.ascontiguousarray((bo / NCORES).reshape(8, 128).T),
            "onesd": _ONES,
        })

    kwargs = {}
    if TRACE:
        try:
            import ntff_shim
            ntff_shim.install()
            kwargs["trace"] = True
        except Exception:
            pass

    res = bass_utils.run_bass_kernel_spmd(
        nc, in_maps, core_ids=list(range(NCORES)), **kwargs)
    LAST_EXEC_NS = res.exec_time_ns

    total = res.results[0]["out"].astype(np.float32).copy()
    for c in range(1, NCORES):
        total += res.results[c]["out"]
    return np.ascontiguousarray(total.T).reshape(B, S, DM)
